# revision 1
# baseline (speedup 1.0000x reference)
"""AdaptiveAttention kernel for 8 trn2 NeuronCores.

Strategy (per sharding hint): data-parallel over batch B=8, one batch
element per NeuronCore; small weights + interpolated pos table are
replicated. All heavy compute (qkv matmul, per-token 8x8 head
attention, scrambled reshape, proj matmul) runs on-device per core;
the host only interpolates the pos table (resolution-dependent weight
preprocessing), shards, and gathers.

Hardcoded problem shape: B=8, N=4096, C=512, H=8, D=64, resolution=16.
"""

import numpy as np

B, N, C = 8, 4096, 512
NUM_HEADS = 8
HEAD_DIM = C // NUM_HEADS

_COMPILED = {}


def _interp_linear_np(pos, out_len):
    # F.interpolate(mode='linear', align_corners=False) along axis 1.
    in_len = pos.shape[1]
    if in_len == out_len:
        return pos
    scale = in_len / out_len
    coords = (np.arange(out_len, dtype=np.float64) + 0.5) * scale - 0.5
    coords = np.clip(coords, 0.0, in_len - 1)
    i0 = np.floor(coords).astype(np.int64)
    i1 = np.minimum(i0 + 1, in_len - 1)
    w = (coords - i0).astype(np.float32)[None, :, None]
    return pos[:, i0, :] * (1.0 - w) + pos[:, i1, :] * w


def _get_pmapped(n_dev):
    key = n_dev
    if key in _COMPILED:
        return _COMPILED[key]

    import jax
    import jax.numpy as jnp

    def per_core(xb, pos, w_qkv, b_qkv, w_proj, b_proj):
        # xb: [N, C] one batch element on this core.
        xb = xb + pos
        qkv = xb @ w_qkv + b_qkv  # [N, 3C]
        qkv = qkv.reshape(N, 3, NUM_HEADS, HEAD_DIM)
        q, k, v = qkv[:, 0], qkv[:, 1], qkv[:, 2]  # [N, H, D]
        # Per-token head-axis attention: [N, H, H]
        attn = jnp.einsum("nid,njd->nij", q, k) / np.sqrt(HEAD_DIM)
        attn = jax.nn.softmax(attn, axis=-1)
        out = jnp.einsum("nij,njd->nid", attn, v)  # [N, H, D]
        # Faithful scrambled flatten: [N,H,D] -> [H,N,D] -> [N, C]
        y = jnp.transpose(out, (1, 0, 2)).reshape(N, C)
        return y @ w_proj + b_proj

    fn = jax.pmap(per_core, in_axes=(0, None, None, None, None, None))
    _COMPILED[key] = fn
    return fn


def kernel(x, pos_32, w_qkv, b_qkv, w_proj, b_proj, resolution):
    x = np.asarray(x, dtype=np.float32)
    pos_32 = np.asarray(pos_32, dtype=np.float32)
    w_qkv = np.asarray(w_qkv, dtype=np.float32)
    b_qkv = np.asarray(b_qkv, dtype=np.float32)
    w_proj = np.asarray(w_proj, dtype=np.float32)
    b_proj = np.asarray(b_proj, dtype=np.float32)

    Bx, Nx, Cx = x.shape
    target_len = int(resolution) ** 3
    pos = _interp_linear_np(pos_32, target_len)

    add_pos = pos.shape[1] == Nx
    pos2d = pos[0] if add_pos else np.zeros((Nx, Cx), np.float32)

    fn = _get_pmapped(Bx)
    out = fn(x, pos2d, w_qkv, b_qkv, w_proj, b_proj)
    return np.asarray(out, dtype=np.float32).reshape(Bx, Nx, Cx)
